# revision 49
# baseline (speedup 1.0000x reference)
"""Trainium2 Bass kernel for nn_ContrastiveLoss_81381040325084.

Reference semantics (fp32):
    y_flat = y.reshape(T*Q, D)                      # column j uses y[j//Q, j%Q]
    S      = exp((x @ y_flat.T) / TEMP)             # [N, T*Q]
    match[i, j] = (track_idxs[i] == j % T)          # y_idxs = tile(arange(T), Q)
    num = sum(S[match]); den = sum(S[~match])
    loss = -log(num / (den + num)) = -log(num / total)

Strategy (8 NeuronCores, data-parallel over rows of x):
  * Host: sort rows of x by track id (16 rows per track for this input), and
    permute columns of y_flat so device column t*Q+q holds y_flat[t + T*q]
    (the column whose label y_idxs == t). Matched columns for track t are then
    the 8 contiguous device columns [t*8, t*8+8).
  * Each core gets 1024 rows = 64 tracks. Its yT copy is rolled so its own 64
    tracks' columns (a 512-wide group) sit at columns [0, 512). For row-block b
    (128 rows = 8 tracks x 16 rows), the matched entries form a static
    [128, 64] block-diagonal mask at columns [b*64, (b+1)*64).
  * x / y are cast to fp8 e4m3 on the host: quarter of the f32 DMA bytes.
    Per-element input rounding error (~2-3%) averages out over the 33.5M
    exp-sum terms and the num/tot bias cancels in the ratio: measured loss
    error vs the f32 reference is ~7e-8.
  * The kernel is ACT(exp)-bound in steady state (ScalarE is the only exp
    engine, 1 elem/lane/cycle @1.2GHz: 16 x [128,2048] ACTIVATEs ~= 33.5us
    busy per core, with accum_out giving the per-row sums for free). The
    optimization targets are the head (time to the first exp) and the tail:
      - input DMAs are spread over the sync/gpsimd/scalar queues; the first
        transfer on each queue pays ~3us of DGE start latency, so the three
        first-position slots carry exactly the chunks the first matmuls
        need (y[0:256], y[256:512], x block 0), and the first matmul is
        split 2x256 to chase them.
      - a PE warm-up on a memset tile (no DMA dependency) ramps the
        HAM-throttled PE clock while inputs land.
      - per-tile tot accums (ACT accumulator reads) and per-block num
        sums (DVE mask-mul + reduce) are written into one padded [128,32]
        tile; the 24 early-final columns DMA out overlapping the last
        tile's compute and the last two ship right after the final
        accumulator read. Host sums the columns and takes -log in f64.
"""

import numpy as np
from contextlib import ExitStack

import ml_dtypes

import concourse.bass as bass
import concourse.tile as tile
from concourse import bacc, mybir
from concourse.bass_utils import run_bass_kernel_spmd

N, T, Q, D = 8192, 512, 8, 128
TEMP = 0.3
NCORES = 8
RPC = N // NCORES            # 1024 rows per core
NB = RPC // 128              # 8 row blocks per core
F32 = mybir.dt.float32
FP8 = mybir.dt.float8e4
NP_FP8 = ml_dtypes.float8_e4m3
MM_N = 512                   # matmul free size (PSUM: one bank per matmul)
NACT = 2 * NB + 1            # one ACT per tile; the first tile is split in 2
# fp8 DoubleRow matmuls (2 contraction rows/cycle) measured ~2.5us SLOWER
# end-to-end: it disables FWL and the LDWEIGHTS penalty dominates at
# stationary width 128. Keep plain fp8 matmuls.
USE_DR = False

_PROG = None


def _build_program():
    nc = bacc.Bacc(
        "TRN2", target_bir_lowering=False, debug=False, num_devices=NCORES
    )
    if USE_DR:
        # DoubleRow: contraction rows packed in pairs, partition dim 64;
        # (p, k) holds D-row 2p+k. Same layout for both operands keeps the
        # contraction a plain permutation of the D axis.
        xT = nc.dram_tensor("xT", [D // 2, 2, RPC], FP8, kind="ExternalInput")
        yT = nc.dram_tensor("yT", [D // 2, 2, T * Q], FP8, kind="ExternalInput")
    else:
        xT = nc.dram_tensor("xT", [D, RPC], FP8, kind="ExternalInput")
        yT = nc.dram_tensor("yT", [D, T * Q], FP8, kind="ExternalInput")
    msk = nc.dram_tensor("msk", [128, 64], F32, kind="ExternalInput")
    out2 = nc.dram_tensor("out2", [128, 32], F32, kind="ExternalOutput")

    inv_t = float(1.0 / TEMP)
    ADD = mybir.AluOpType.add
    MUL = mybir.AluOpType.mult

    with tile.TileContext(nc) as tc, ExitStack() as ctx:
        ypool = ctx.enter_context(tc.tile_pool(name="ypool", bufs=1))
        cpool = ctx.enter_context(tc.tile_pool(name="cpool", bufs=1))
        pspool = ctx.enter_context(
            tc.tile_pool(name="pspool", bufs=2, space=bass.MemorySpace.PSUM)
        )
        scpool = ctx.enter_context(tc.tile_pool(name="scpool", bufs=2))

        if USE_DR:
            yt_ = ypool.tile([D // 2, 2, T * Q], FP8, tag="y")
            xt_ = cpool.tile([D // 2, 2, RPC], FP8, tag="x")

            def ysl(a, b):
                return yt_[:, :, a:b]

            def xsl(a, b):
                return xt_[:, :, a:b]
        else:
            yt_ = ypool.tile([D, T * Q], FP8, tag="y")
            xt_ = cpool.tile([D, RPC], FP8, tag="x")

            def ysl(a, b):
                return yt_[:, a:b]

            def xsl(a, b):
                return xt_[:, a:b]
        mask_t = cpool.tile([128, 64], F32, tag="mask")
        warm_t = cpool.tile([128, 128], F32, tag="warm")
        # single padded output tile; layout puts everything that is final by
        # ~2 tiles before the end in cols 0:24 (bulk DMA overlapping the last
        # tile) and the last two ACT tiles' tot accums in cols 24:26 (tiny
        # final DMA): tot for ACT-instr i -> col i (i < NACT-2) else
        # 24+(i-(NACT-2)); num block b -> col 16+b; the rest stays zero
        # padding. Host sums the columns.
        po_t = cpool.tile([128, 32], F32, tag="po")

        def tot_col(i):
            return i if i < NACT - 2 else 24 + (i - (NACT - 2))

        # PE warm-up fuel with no DMA dependency: DVE memset, then dummy
        # matmuls below ramp the HAM-throttled PE clock while inputs land.
        nc.vector.memset(warm_t[:], 0.0)
        nc.vector.memset(po_t[:], 0.0)
        warm8 = warm_t[:].bitcast(FP8)  # [128, 512] fp8 zeros

        def ydram(a, b):
            return yT[:, :, a:b] if USE_DR else yT[:, a:b]

        def xdram(a, b):
            return xT[:, :, a:b] if USE_DR else xT[:, a:b]

        # Input DMAs spread across the three DMA-capable queues. The first
        # transfer on each queue pays ~3us of DGE start latency, so the
        # three first-position slots carry the three chunks the first
        # matmuls need (y[0:256], y[256:512], x block 0); everything else
        # follows in need-order.
        nc.sync.dma_start(ysl(0, 256), ydram(0, 256))
        nc.gpsimd.dma_start(xsl(0, 128), xdram(0, 128))
        nc.scalar.dma_start(ysl(256, 512), ydram(256, 512))
        nc.gpsimd.dma_start(ysl(512, 1024), ydram(512, 1024))
        nc.sync.dma_start(ysl(1024, 1536), ydram(1024, 1536))
        nc.scalar.dma_start(mask_t[:], msk[:])
        nc.scalar.dma_start(ysl(1536, 2048), ydram(1536, 2048))
        nc.gpsimd.dma_start(ysl(3072, 4096), ydram(3072, 4096))
        nc.sync.dma_start(ysl(2048, 3072), ydram(2048, 3072))
        nc.sync.dma_start(xsl(128, 512), xdram(128, 512))
        nc.sync.dma_start(xsl(512, RPC), xdram(512, RPC))

        # PE warm-up: narrow dummy matmuls on the memset tile ramp the
        # HAM-throttled PE clock while inputs land; sized to keep the PE
        # busy until the first real matmul's inputs arrive.
        warm_ps = pspool.tile([128, 2048], F32, tag="ps")
        for _ in range(16):
            nc.tensor.matmul(
                warm_ps[:, 0:128],
                warm8[:, 0:128],
                warm8[:, 128:256],
                start=True,
                stop=True,
            )

        def act_exp(ps_ap, col):
            nc.scalar.activation(
                ps_ap,
                ps_ap,
                mybir.ActivationFunctionType.Exp,
                scale=inv_t,
                accum_out=po_t[:, col : col + 1],
            )

        def num_reduce(ps, b):
            sc = scpool.tile([128, 64], F32, tag="sc")
            nc.vector.tensor_mul(sc[:], ps[:, b * 64 : (b + 1) * 64], mask_t[:])
            nc.vector.tensor_reduce(
                po_t[:, 16 + b : 17 + b], sc[:], axis=mybir.AxisListType.X, op=ADD
            )

        perf_mode = mybir.MatmulPerfMode.DoubleRow if USE_DR else None

        def mm(ps_ap, xb, a, b):
            nc.tensor.matmul(
                ps_ap, xb, ysl(a, b), start=True, stop=True, perf_mode=perf_mode
            )

        # The first [128,2048] tile of row-block 0 is processed as two
        # separate half-tiles (each in its own PSUM slot) so the first exp
        # can start after only 1024 y columns' matmuls instead of 2048 —
        # ~1.3us earlier given the DMA-landing pace at the head. The first
        # matmul is further split 2x256 to chase the first two y chunks.
        xb0 = xsl(0, 128)
        psA = pspool.tile([128, 2048], F32, tag="ps")
        mm(psA[:, 0:256], xb0, 0, 256)
        mm(psA[:, 256:512], xb0, 256, 512)
        mm(psA[:, 512:1024], xb0, 512, 1024)
        act_exp(psA[:, 0:1024], tot_col(0))
        num_reduce(psA, 0)
        psB = pspool.tile([128, 2048], F32, tag="ps")
        mm(psB[:, 0:512], xb0, 1024, 1536)
        mm(psB[:, 512:1024], xb0, 1536, 2048)
        act_exp(psB[:, 0:1024], tot_col(1))

        act_i = 2
        for b in range(NB):
            xb = xsl(b * 128, (b + 1) * 128)
            for h in range(2):
                if b == 0 and h == 0:
                    continue  # handled by the split head tiles above
                ps = pspool.tile([128, 2048], F32, tag="ps")
                for g in range(2048 // MM_N):
                    c0 = h * 2048 + g * MM_N
                    mm(ps[:, g * MM_N : (g + 1) * MM_N], xb, c0, c0 + MM_N)
                act_exp(ps[:], tot_col(act_i))
                act_i += 1
                if h == 0:
                    num_reduce(ps, b)

        # bulk output DMA (cols final by ~2 tiles before the end) overlaps
        # the last tile's compute; the last two tot columns ship from the
        # scalar queue right after the final accumulator read.
        nc.sync.dma_start(out2[:, 0:24], po_t[:, 0:24])
        nc.scalar.dma_start(out2[:, 24:32], po_t[:, 24:32])
    nc.compile()
    return nc


def get_program():
    global _PROG
    if _PROG is None:
        _PROG = _build_program()
    return _PROG


def make_in_maps(x, y):
    """Build per-core input maps from full x [N, D] (already track-sorted,
    f32) and y [T, Q, D] (f32)."""
    yf = np.ascontiguousarray(y, dtype=np.float32).reshape(T * Q, D)
    # device column t*Q+q  <-  y_flat[t + T*q]  (label-major ordering)
    ycols = np.ascontiguousarray(yf.reshape(Q, T, D).transpose(1, 0, 2)).reshape(
        T * Q, D
    )
    yT_full = np.ascontiguousarray(ycols.T)  # [D, T*Q] f32
    # rows per track = N//T = 16; block = 8 tracks x 16 rows; mask[p, c] =
    # (c//8 == p//16)
    mask = (
        np.arange(64)[None, :] // Q == np.arange(128)[:, None] // (N // T)
    ).astype(np.float32)
    in_maps = []
    for c in range(NCORES):
        xc = x[c * RPC : (c + 1) * RPC]  # [RPC, D]
        xTc = np.ascontiguousarray(xc.T).astype(NP_FP8)  # [D, RPC]
        yTc = np.ascontiguousarray(np.roll(yT_full, -c * 512, axis=1)).astype(
            NP_FP8
        )
        if USE_DR:
            xTc = xTc.reshape(D // 2, 2, RPC)
            yTc = yTc.reshape(D // 2, 2, T * Q)
        in_maps.append({"xT": xTc, "yT": yTc, "msk": mask})
    return in_maps


def _reduce_results(results):
    tot = np.float64(0.0)
    num = np.float64(0.0)
    for r in results:
        o2 = r["out2"].astype(np.float64)
        # tot ACT-instr i -> col i (i<NACT-2) else 24+...; num b -> col 16+b
        tot += o2[:, 0 : NACT - 2].sum() + o2[:, 24:26].sum()
        num += o2[:, 16:24].sum()  # remaining cols are zero padding
    loss = -np.log(num / tot)
    return np.array([loss], dtype=np.float32)


def _kernel_numpy_fallback(x, track_idxs, y):
    """Pure-host fallback for inputs without exactly N/T rows per track."""
    yf = y.astype(np.float64).reshape(T * Q, D)
    yidx = np.tile(np.arange(T), Q)
    tot = np.float64(0.0)
    num = np.float64(0.0)
    for i0 in range(0, N, 512):
        S = np.exp(x[i0 : i0 + 512].astype(np.float64) @ yf.T / TEMP)
        m = track_idxs[i0 : i0 + 512, None] == yidx[None, :]
        tot += S.sum()
        num += S[m].sum()
    return np.array([-np.log(num / tot)], dtype=np.float32)


def kernel(x, track_idxs, y):
    x = np.ascontiguousarray(np.asarray(x), dtype=np.float32)
    y = np.ascontiguousarray(np.asarray(y), dtype=np.float32)
    ti = np.asarray(track_idxs).astype(np.int64)
    if not np.all(np.bincount(ti, minlength=T) == N // T):
        return _kernel_numpy_fallback(x, ti, y)
    perm = np.argsort(ti, kind="stable")  # rows grouped by track id
    xs = np.ascontiguousarray(x[perm])
    in_maps = make_in_maps(xs, y)
    nc = get_program()
    res = run_bass_kernel_spmd(nc, in_maps, list(range(NCORES))).results
    return _reduce_results(res)
